# revision 3
# baseline (speedup 1.0000x reference)
"""Trainium2 Bass kernel for the 2-branch GCN+GAT+GraphNorm network (v2).

Design (8 NeuronCores, SPMD):
  - Nodes rebalanced into 784 blocks of 128 (snake-deal by degree) so each
    block owns <= RT*128 incident edges; core c owns blocks b with b%8==c.
  - Edge aggregation per block: 4 region dma_gathers (int16 idx, 512B rows)
    pull h[src] rows; one-hot S (built on DVE) turns segment-sum into PE
    matmuls accumulating in PSUM. GAT softmax folded into post-division.
  - a_s[src] rides in the gathered row; a_d[dst] expanded per edge via a
    preloaded fp8 one-hot-transpose (ST) matmul against resident local a_d.
  - Aggregates live in SBUF (no DRAM bounce for edge->node handoff).
  - GraphNorm via per-graph one-hot moment matmuls + one AllReduce per norm.
  - Node features exchanged by AllGather of bf16 bounce rows (double-buffered
    hfull tables). Branches interleaved so collectives/node phases hide under
    the other branch's edge phase.
"""
import numpy as np
import ml_dtypes

P = 128
NCORES = 8
G = 64
H = 4
EPS = 1e-5
NR = 4           # gather regions (int16 index limit)
WROW = 136       # hfull row width (bf16 cols): h | a_s | a_d


# ----------------------------------------------------------------------------
# Host-side graph preprocessing
# ----------------------------------------------------------------------------

def _balance_nodes(deg, nbins):
    """Snake-deal nodes (desc degree) into bins of exactly P slots."""
    n = len(deg)
    order = np.argsort(-deg, kind="stable")
    bin_of = np.empty(n, np.int32)
    nfull = n // nbins
    for r in range(nfull):
        row = order[r * nbins:(r + 1) * nbins]
        if r % 2 == 0:
            bin_of[row] = np.arange(nbins)
        else:
            bin_of[row] = nbins - 1 - np.arange(nbins)
    rem = n - nfull * nbins
    if rem:
        row = order[nfull * nbins:]
        loads = np.bincount(bin_of[order[:nfull * nbins]],
                            weights=deg[order[:nfull * nbins]],
                            minlength=nbins)
        cnts = np.bincount(bin_of[order[:nfull * nbins]], minlength=nbins)
        free = np.where(cnts < P)[0]
        tgt = free[np.argsort(loads[free], kind="stable")][:rem]
        bin_of[row] = tgt[np.argsort(-deg[row], kind="stable")]
    slot_of = np.empty(n, np.int32)
    for b_ in range(nbins):
        nodes = np.where(bin_of == b_)[0]
        assert len(nodes) <= P
        slot_of[nodes] = np.arange(len(nodes))
    return bin_of, slot_of


def prep_branch(x, edge_index, batch):
    n = x.shape[0]
    f_in = x.shape[1]
    nloc = n // NCORES
    nt = (nloc + P - 1) // P
    nloc_pad = nt * P
    nbins = nt * NCORES
    nrows = nloc_pad * NCORES
    rgs = nrows // NR
    assert nrows % NR == 0 and rgs <= 32767

    src = np.concatenate([edge_index[0].astype(np.int64),
                          np.arange(n, dtype=np.int64)])
    dst = np.concatenate([edge_index[1].astype(np.int64),
                          np.arange(n, dtype=np.int64)])
    deg = np.bincount(dst, minlength=n).astype(np.float64)
    dis = 1.0 / np.sqrt(np.maximum(deg, 1.0))

    bin_of, slot_of = _balance_nodes(deg, nbins)
    core_of = (bin_of % NCORES).astype(np.int32)
    t_of = (bin_of // NCORES).astype(np.int32)
    grow = core_of.astype(np.int64) * nloc_pad + t_of * P + slot_of

    ebin = bin_of[dst]
    cnt_b = np.bincount(ebin, minlength=nbins)
    rq = int(np.ceil(cnt_b.max() / P))           # tiles per block

    pk = dict(n=n, f_in=f_in, nloc=nloc, nt=nt, nloc_pad=nloc_pad,
              nrows=nrows, rgs=rgs, rq=rq,
              src=src, dst=dst, ebin=ebin, key=ebin.astype(np.int64),
              grow=grow, dis=dis, core_of=core_of, t_of=t_of,
              slot_of=slot_of, batch=np.asarray(batch), x=x)
    return pk


def pack_branch(pk, rq):
    """Build per-core arrays given the (global) region quota rq."""
    n, nt, nloc_pad = pk["n"], pk["nt"], pk["nloc_pad"]
    rt = rq
    ecap = rt * P
    nbins = nt * NCORES
    src, dst = pk["src"], pk["dst"]
    grow, slot_of = pk["grow"], pk["slot_of"]
    core_of, t_of = pk["core_of"], pk["t_of"]
    batch, dis, x = pk["batch"], pk["dis"], pk["x"]

    # sort edges by (bin, region)
    order = np.argsort(pk["key"], kind="stable")
    es, ed = src[order], dst[order]
    kk = pk["key"][order]
    cnt = np.bincount(kk, minlength=nbins * NR)
    start = np.zeros(nbins * NR + 1, np.int64)
    np.cumsum(cnt, out=start[1:])

    idxg = np.zeros((NCORES, P, nt, rt), np.int32)
    dstf = np.full((NCORES, P, nt, rt), -1.0, np.float32)
    st = np.zeros((NCORES, P, nt, rt * P), np.float32)
    for b_ in range(nbins):
        c, t = b_ % NCORES, b_ // NCORES
        ne = cnt[b_]
        assert ne <= ecap
        lo = start[b_]
        sl = np.arange(ne)
        p_, j_ = (sl % P).astype(np.int64), sl // P
        idxg[c, p_, t, j_] = grow[es[lo:lo + ne]].astype(np.int32)
        dsl = slot_of[ed[lo:lo + ne]]
        dstf[c, p_, t, j_] = dsl
        st[c, dsl, t, j_ * P + p_] = 1.0

    batf = np.full((NCORES, P, nt), -1.0, np.float32)
    disf = np.zeros((NCORES, P, nt), np.float32)
    xT = np.zeros((NCORES, P, nt, P), np.float32)
    bt = np.zeros((NCORES, P, nt, P), np.float32)
    nodes = np.arange(n)
    batf[core_of, slot_of, t_of] = batch.astype(np.float32)
    disf[core_of, slot_of, t_of] = dis.astype(np.float32)
    xT[core_of[:, None], np.arange(pk["f_in"])[None, :], t_of[:, None],
       slot_of[:, None]] = x[nodes]
    bt[core_of, batch.astype(np.int64), t_of, slot_of] = 1.0

    arrs = dict(
        idxg=idxg,
        dstf=dstf.astype(ml_dtypes.bfloat16),
        st=st.astype(ml_dtypes.float8_e4m3fn),
        batf=batf.astype(ml_dtypes.bfloat16),
        disf=disf,
        xT=xT.astype(ml_dtypes.bfloat16),
        bt=bt.astype(ml_dtypes.bfloat16),
    )
    return arrs, rt


# ----------------------------------------------------------------------------
# Device program
# ----------------------------------------------------------------------------

def _build_program(nt, nloc_pad, nrows, rgs, rq, f, f_out, consts):
    import concourse.bass as bass
    import concourse.mybir as mybir
    import concourse.tile as tile
    import concourse.bacc as bacc
    import contextlib

    fp32 = mybir.dt.float32
    bf16 = mybir.dt.bfloat16
    fp8 = mybir.dt.float8e4
    i32 = mybir.dt.int32
    AF = mybir.ActivationFunctionType
    OP = mybir.AluOpType
    ds = bass.ds
    C = f // H
    RT = rq
    rg = [list(range(NCORES))]

    nc = bacc.Bacc("TRN2", target_bir_lowering=False, debug=False,
                   num_devices=NCORES)

    ext = {}
    for b in (1, 2):
        ext[f"xT{b}"] = nc.dram_tensor(f"xT{b}", [P, nt, P], bf16, kind="ExternalInput")
        ext[f"idx{b}"] = nc.dram_tensor(f"idx{b}", [P, nt, RT], i32, kind="ExternalInput")
        ext[f"dstf{b}"] = nc.dram_tensor(f"dstf{b}", [P, nt, RT], bf16, kind="ExternalInput")
        ext[f"st{b}"] = nc.dram_tensor(f"st{b}", [P, nt, RT * P], fp8, kind="ExternalInput")
        ext[f"bt{b}"] = nc.dram_tensor(f"bt{b}", [P, nt, P], bf16, kind="ExternalInput")
        ext[f"batf{b}"] = nc.dram_tensor(f"batf{b}", [P, nt], bf16, kind="ExternalInput")
        ext[f"disf{b}"] = nc.dram_tensor(f"disf{b}", [P, nt], fp32, kind="ExternalInput")
        ext[f"out{b}"] = nc.dram_tensor(f"out{b}", [nloc_pad, f_out], fp32, kind="ExternalOutput")

    dram = {}
    for b in (1, 2):
        dram[f"bounce{b}"] = nc.dram_tensor(f"bounce{b}", [nloc_pad, WROW], bf16)
        for k in (0, 1):
            dram[f"hfull{b}{k}"] = nc.dram_tensor(
                f"hfull{b}{k}", [nrows, WROW], bf16, addr_space="Shared")
        for l in range(3):
            dram[f"arin{b}{l}"] = nc.dram_tensor(f"arin{b}{l}", [P, 2 * f], fp32)
            dram[f"arout{b}{l}"] = nc.dram_tensor(f"arout{b}{l}", [P, 2 * f], fp32,
                                                  addr_space="Shared")

    cd = {}
    for k, v in consts.items():
        cd[k] = nc.inline_tensor(np.ascontiguousarray(v), name=f"c_{k}")

    with tile.TileContext(nc) as tc:
        stack = contextlib.ExitStack()
        sb_const = stack.enter_context(tc.tile_pool(name="const", bufs=1))
        sb_big = stack.enter_context(tc.tile_pool(name="big", bufs=1))
        ep = stack.enter_context(tc.tile_pool(name="edge", bufs=2))
        gp = stack.enter_context(tc.tile_pool(name="gath", bufs=2))
        np_ = stack.enter_context(tc.tile_pool(name="node", bufs=2))
        mp = stack.enter_context(tc.tile_pool(name="mid", bufs=1))
        ps_num = stack.enter_context(tc.tile_pool(name="pnum", bufs=2, space="PSUM"))
        ps_pade = stack.enter_context(tc.tile_pool(name="ppade", bufs=2, space="PSUM"))
        ps_a = stack.enter_context(tc.tile_pool(name="pa", bufs=1, space="PSUM"))
        ps_st = stack.enter_context(tc.tile_pool(name="pst", bufs=1, space="PSUM"))
        ps_bc = stack.enter_context(tc.tile_pool(name="pbc", bufs=1, space="PSUM"))

        cs = {}
        for k, t in cd.items():
            cs[k] = sb_const.tile(list(t.shape), t.dtype, name=f"s_{k}")
            nc.sync.dma_start(out=cs[k][:], in_=t[:])

        x_sb, agg_sb, ad_sb, bat_sb, dis_sb, stats_sb, nstats_sb = {}, {}, {}, {}, {}, {}, {}
        for b in (1, 2):
            x_sb[b] = sb_big.tile([P, nt, f], bf16, name=f"x_sb{b}")
            agg_sb[b] = sb_big.tile([P, nt, f], bf16, name=f"agg_sb{b}")
            ad_sb[b] = sb_big.tile([P, nt, H], bf16, name=f"ad_sb{b}")
            bat_sb[b] = sb_big.tile([P, nt], bf16, name=f"bat_sb{b}")
            nc.sync.dma_start(out=bat_sb[b][:], in_=ext[f"batf{b}"][:])
            dis_sb[b] = sb_big.tile([P, nt], fp32, name=f"dis_sb{b}")
            nc.sync.dma_start(out=dis_sb[b][:], in_=ext[f"disf{b}"][:])
            stats_sb[b] = sb_big.tile([P, 2 * f], fp32, name=f"stats_sb{b}")
            nstats_sb[b] = sb_big.tile([P, 2 * f], fp32, name=f"nstats_sb{b}")

        def nrow(dt_, t, w0, w1):
            return dt_[:].rearrange("(t p) w -> p t w", p=P)[:, ds(t, 1), w0:w1] \
                .rearrange("p a w -> p (a w)")

        def xv(sb, b, t, w=None):
            v = sb[b][:][:, ds(t, 1), :]
            return v.rearrange("p a w -> p (a w)")

        # ---------------- n0: h0' = dis * (x @ W0) -> bounce ----------------
        def phase_n0(b):
            with tc.For_i(0, nt, 1) as t:
                xt = np_.tile([P, f], bf16, tag="xt")
                nc.sync.dma_start(
                    out=xt[:],
                    in_=ext[f"xT{b}"][:, ds(t, 1), :].rearrange("p a w -> p (a w)"))
                pha = ps_a.tile([P, f], fp32, tag="pha")
                nc.tensor.matmul(out=pha[:], lhsT=xt[:], rhs=cs["mov0"][:],
                                 start=True, stop=True, skip_group_check=True)
                hb = np_.tile([P, f], bf16, tag="hb")
                nc.vector.tensor_scalar_mul(out=hb[:], in0=pha[:],
                                            scalar1=dis_sb[b][:, ds(t, 1)])
                nc.sync.dma_start(out=nrow(dram[f"bounce{b}"], t, 0, f), in_=hb[:])

        def ag(b, k):
            nc.gpsimd.collective_compute(
                "AllGather", mybir.AluOpType.bypass, replica_groups=rg,
                ins=[dram[f"bounce{b}"][:]], outs=[dram[f"hfull{b}{k}"][:]])

        # ---------------- edge phase ----------------
        def phase_edge(b, l):
            gat = l > 0
            k = l % 2
            hf = dram[f"hfull{b}{k}"]
            with tc.For_i(0, nt, 1) as blk:
                idxs = ep.tile([P, RT], i32, tag="idxs")
                nc.sync.dma_start(
                    out=idxs[:],
                    in_=ext[f"idx{b}"][:, ds(blk, 1), :].rearrange("p a w -> p (a w)"))
                dstf_t = ep.tile([P, RT], bf16, tag="dstf")
                nc.sync.dma_start(
                    out=dstf_t[:],
                    in_=ext[f"dstf{b}"][:, ds(blk, 1), :].rearrange("p a w -> p (a w)"))
                gw = WROW if gat else f
                Gt = gp.tile([P, RT, gw], bf16, tag="G")
                for j in range(RT):
                    nc.gpsimd.indirect_dma_start(
                        out=Gt[:, j, :], out_offset=None, in_=hf[:],
                        in_offset=bass.IndirectOffsetOnAxis(ap=idxs[:, j:j + 1],
                                                            axis=0))
                S_all = ep.tile([P, RT, P], bf16, tag="S")
                nc.vector.tensor_tensor(
                    out=S_all[:],
                    in0=dstf_t[:].rearrange("p (t o) -> p t o", o=1).broadcast_to([P, RT, P]),
                    in1=cs["iota"][:].rearrange("p (o e) -> p o e", o=1).broadcast_to([P, RT, P]),
                    op=OP.is_equal)
                if gat:
                    st_t = ep.tile([P, RT * P], fp8, tag="st")
                    nc.sync.dma_start(
                        out=st_t[:],
                        in_=ext[f"st{b}"][:, ds(blk, 1), :].rearrange("p a w -> p (a w)"))
                    adb = ep.tile([P, H], bf16, tag="adb")
                    nc.vector.tensor_copy(out=adb[:], in_=xv(ad_sb, b, blk))
                    pade = ps_pade.tile([P, RT * H], fp32, tag="pade")
                    for j in range(RT):
                        nc.tensor.matmul(out=pade[:, j * H:(j + 1) * H],
                                         lhsT=st_t[:, j * P:(j + 1) * P], rhs=adb[:],
                                         start=True, stop=True, skip_group_check=True)
                    eatt = ep.tile([P, RT, H], bf16, tag="eatt")
                    nc.vector.tensor_add(
                        out=eatt[:],
                        in0=Gt[:, :, f:f + H],
                        in1=pade[:].rearrange("p (t h) -> p t h", h=H))
                    lr = ep.tile([P, RT, H], bf16, tag="lr")
                    nc.vector.tensor_scalar_mul(out=lr[:], in0=eatt[:], scalar1=0.2)
                    nc.vector.tensor_max(out=lr[:], in0=lr[:], in1=eatt[:])
                    ex = ep.tile([P, RT, H], bf16, tag="ex")
                    nc.scalar.activation(out=ex[:], in_=lr[:], func=AF.Exp)
                    wex = ep.tile([P, RT, f + H], bf16, tag="wex")
                    exv = ex[:]
                    for hh in range(H):
                        nc.vector.tensor_mul(
                            out=wex[:, :, hh * C:(hh + 1) * C],
                            in0=Gt[:, :, hh * C:(hh + 1) * C],
                            in1=exv[:, :, hh:hh + 1].broadcast_to([P, RT, C]))
                    nc.vector.tensor_copy(out=wex[:, :, f:f + H], in_=exv)
                    pnum = ps_num.tile([P, f + H], fp32, tag="num")
                    for j in range(RT):
                        nc.tensor.matmul(out=pnum[:], lhsT=S_all[:, j, :],
                                         rhs=wex[:, j, :],
                                         start=(j == 0), stop=(j == RT - 1),
                                         skip_group_check=True)
                    den = ep.tile([P, H], fp32, tag="den")
                    nc.vector.tensor_scalar_add(out=den[:], in0=pnum[:, f:f + H],
                                                scalar1=1e-16)
                    denr = ep.tile([P, H], fp32, tag="denr")
                    nc.vector.reciprocal(out=denr[:], in_=den[:])
                    nc.vector.tensor_mul(
                        out=agg_sb[b][:][:, ds(blk, 1), :]
                            .rearrange("p a (h c) -> p (a h) c", c=C),
                        in0=pnum[:, :f].rearrange("p (h c) -> p h c", c=C),
                        in1=denr[:].rearrange("p (h o) -> p h o", o=1)
                            .broadcast_to([P, H, C]))
                else:
                    pnum = ps_num.tile([P, f], fp32, tag="num")
                    for j in range(RT):
                        nc.tensor.matmul(out=pnum[:], lhsT=S_all[:, j, :],
                                         rhs=Gt[:, j, :],
                                         start=(j == 0), stop=(j == RT - 1),
                                         skip_group_check=True)
                    nc.vector.tensor_scalar_mul(
                        out=xv(agg_sb, b, blk), in0=pnum[:],
                        scalar1=dis_sb[b][:, ds(blk, 1)])

        # ---------------- node phase ----------------
        def phase_node(b, l):
            gat = l > 0
            ln = ["gn0", "gn1", "gn2"][l]
            nc.vector.memset(stats_sb[b][:], 0.0)
            with tc.For_i(0, nt, 1) as t:
                av = xv(agg_sb, b, t)
                if gat:
                    hpre = np_.tile([P, f], bf16, tag="hpre")
                    nc.vector.tensor_add(out=hpre[:], in0=av, in1=cs[f"brow{l}"][:])
                    nc.scalar.activation(out=av, in_=hpre[:], func=AF.Gelu)
                else:
                    nc.vector.tensor_add(out=av, in0=av, in1=cs["brow0"][:])
                sq = np_.tile([P, f], bf16, tag="sq")
                nc.scalar.activation(out=sq[:], in_=av, func=AF.Square)
                B = np_.tile([P, P], bf16, tag="B")
                nc.vector.tensor_tensor(
                    out=B[:], in0=bat_sb[b][:, ds(t, 1)].to_broadcast([P, P]),
                    in1=cs["iota"][:], op=OP.is_equal)
                pst = ps_st.tile([P, 2 * f], fp32, tag="pst")
                nc.tensor.matmul(out=pst[:, :f], lhsT=B[:], rhs=av,
                                 start=True, stop=True, skip_group_check=True)
                nc.tensor.matmul(out=pst[:, f:], lhsT=B[:], rhs=sq[:],
                                 start=True, stop=True, skip_group_check=True)
                nc.vector.tensor_add(out=stats_sb[b][:], in0=stats_sb[b][:],
                                     in1=pst[:])
            # AllReduce moments
            nc.sync.dma_start(out=dram[f"arin{b}{l}"][:], in_=stats_sb[b][:])
            nc.gpsimd.collective_compute(
                "AllReduce", mybir.AluOpType.add, replica_groups=rg,
                ins=[dram[f"arin{b}{l}"][:]], outs=[dram[f"arout{b}{l}"][:]])
            nc.sync.dma_start(out=nstats_sb[b][:], in_=dram[f"arout{b}{l}"][:])
            mean = mp.tile([P, f], fp32, tag="mean")
            nc.vector.tensor_scalar_mul(out=mean[:], in0=nstats_sb[b][:, :f],
                                        scalar1=cs[f"cntinv{b}"][:])
            e2 = mp.tile([P, f], fp32, tag="e2")
            nc.vector.tensor_scalar_mul(out=e2[:], in0=nstats_sb[b][:, f:],
                                        scalar1=cs[f"cntinv{b}"][:])
            m2 = mp.tile([P, f], fp32, tag="m2")
            nc.vector.tensor_mul(out=m2[:], in0=mean[:], in1=mean[:])
            nc.vector.tensor_mul(out=m2[:], in0=m2[:], in1=cs[f"am_{ln}"][:])
            var = mp.tile([P, f], fp32, tag="var")
            nc.vector.tensor_sub(out=var[:], in0=e2[:], in1=m2[:])
            nc.vector.tensor_scalar_add(out=var[:], in0=var[:], scalar1=EPS)
            sd = mp.tile([P, f], fp32, tag="sd")
            nc.scalar.activation(out=sd[:], in_=var[:], func=AF.Sqrt)
            rstd = mp.tile([P, f], fp32, tag="rstd")
            nc.vector.reciprocal(out=rstd[:], in_=sd[:])
            ns2 = mp.tile([P, 2 * f], bf16, tag="ns2")
            nc.vector.tensor_mul(out=ns2[:, :f], in0=mean[:], in1=cs[f"al_{ln}"][:])
            nc.vector.tensor_mul(out=ns2[:, f:], in0=rstd[:], in1=cs[f"ga_{ln}"][:])
            # pass 2
            with tc.For_i(0, nt, 1) as t:
                BT = np_.tile([P, P], bf16, tag="BT")
                nc.sync.dma_start(
                    out=BT[:],
                    in_=ext[f"bt{b}"][:, ds(t, 1), :].rearrange("p a w -> p (a w)"))
                pbc = ps_bc.tile([P, 2 * f], fp32, tag="pbc")
                nc.tensor.matmul(out=pbc[:], lhsT=BT[:], rhs=ns2[:],
                                 start=True, stop=True, skip_group_check=True)
                hv = xv(agg_sb, b, t)
                xc = np_.tile([P, f], bf16, tag="xc")
                nc.vector.tensor_sub(out=xc[:], in0=hv, in1=pbc[:, :f])
                nc.vector.tensor_mul(out=xc[:], in0=xc[:], in1=pbc[:, f:])
                xvw = xv(x_sb, b, t)
                if gat:
                    nc.vector.tensor_add(out=xc[:], in0=xc[:], in1=cs[f"be_{ln}"][:])
                    nc.vector.tensor_add(out=xvw, in0=xvw, in1=xc[:])
                else:
                    nc.vector.tensor_add(out=xvw, in0=xc[:], in1=cs[f"be_{ln}"][:])
                xt = np_.tile([P, f], bf16, tag="xt")
                nc.sync.dma_start_transpose(out=xt[:], in_=xvw)
                if l < 2:
                    pha = ps_a.tile([P, f + 2 * H], fp32, tag="pha")
                    nc.tensor.matmul(out=pha[:], lhsT=xt[:], rhs=cs[f"mov{l + 1}"][:],
                                     start=True, stop=True, skip_group_check=True)
                    hb = np_.tile([P, f + 2 * H], bf16, tag="hb2")
                    nc.scalar.activation(out=hb[:], in_=pha[:], func=AF.Copy)
                    nc.vector.tensor_copy(out=xv(ad_sb, b, t), in_=hb[:, f + H:f + 2 * H])
                    nc.sync.dma_start(out=nrow(dram[f"bounce{b}"], t, 0, f + 2 * H),
                                      in_=hb[:])
                else:
                    pha = ps_a.tile([P, f_out], fp32, tag="phaL")
                    nc.tensor.matmul(out=pha[:], lhsT=xt[:], rhs=cs["movL"][:],
                                     start=True, stop=True, skip_group_check=True)
                    ob = np_.tile([P, f_out], fp32, tag="ob")
                    nc.vector.tensor_add(out=ob[:], in0=pha[:], in1=cs["linb"][:])
                    nc.sync.dma_start(out=nrow(ext[f"out{b}"], t, 0, f_out), in_=ob[:])

        # ---------------- schedule ----------------
        phase_n0(1); ag(1, 0)
        phase_n0(2); ag(2, 0)
        phase_edge(1, 0); phase_node(1, 0); ag(1, 1)
        phase_edge(2, 0); phase_node(2, 0); ag(2, 1)
        phase_edge(1, 1); phase_node(1, 1); ag(1, 0)
        phase_edge(2, 1); phase_node(2, 1); ag(2, 0)
        phase_edge(1, 2); phase_node(1, 2)
        phase_edge(2, 2); phase_node(2, 2)
        stack.close()

    nc.compile()
    return nc


# ----------------------------------------------------------------------------
# Const construction
# ----------------------------------------------------------------------------

def _make_consts(f, f_out, params, cnt1, cnt2):
    C = f // H
    (W0, b0, gn0_gamma, gn0_beta, gn0_alpha, gat_W, gat_att_src, gat_att_dst,
     gat_b, gn_gamma, gn_beta, gn_alpha, lin_W, lin_b) = params
    cons = {}
    cons["iota"] = np.tile(np.arange(P, dtype=np.float32), (P, 1)).astype(ml_dtypes.bfloat16)
    cons["mov0"] = W0.T.astype(ml_dtypes.bfloat16)
    for l in range(2):
        Asm = np.zeros((f, H), np.float32)
        Adm = np.zeros((f, H), np.float32)
        for hh in range(H):
            Asm[hh * C:(hh + 1) * C, hh] = gat_att_src[l][hh]
            Adm[hh * C:(hh + 1) * C, hh] = gat_att_dst[l][hh]
        mv = np.concatenate([gat_W[l].T, gat_W[l].T @ Asm, gat_W[l].T @ Adm], 1)
        cons[f"mov{l + 1}"] = mv.astype(ml_dtypes.bfloat16)
        cons[f"brow{l + 1}"] = np.tile(gat_b[l], (P, 1)).astype(ml_dtypes.bfloat16)
    cons["movL"] = lin_W.T.astype(ml_dtypes.bfloat16)
    cons["linb"] = np.tile(lin_b, (P, 1)).astype(np.float32)
    cons["brow0"] = np.tile(b0, (P, 1)).astype(ml_dtypes.bfloat16)
    for ln, ga, be, al in [("gn0", gn0_gamma, gn0_beta, gn0_alpha),
                           ("gn1", gn_gamma[0], gn_beta[0], gn_alpha[0]),
                           ("gn2", gn_gamma[1], gn_beta[1], gn_alpha[1])]:
        cons[f"ga_{ln}"] = np.tile(ga, (P, 1)).astype(np.float32)
        cons[f"be_{ln}"] = np.tile(be, (P, 1)).astype(ml_dtypes.bfloat16)
        cons[f"al_{ln}"] = np.tile(al, (P, 1)).astype(np.float32)
        cons[f"am_{ln}"] = np.tile(2 * al - al * al, (P, 1)).astype(np.float32)
    cons["cntinv1"] = np.concatenate([1.0 / cnt1, np.ones(P - G, np.float32)])[:, None].astype(np.float32)
    cons["cntinv2"] = np.concatenate([1.0 / cnt2, np.ones(P - G, np.float32)])[:, None].astype(np.float32)
    return cons


# ----------------------------------------------------------------------------
# PJRT runner (reusable jitted executable)
# ----------------------------------------------------------------------------

class _Runner:
    def __init__(self, nc, n_cores):
        import jax
        from jax.sharding import Mesh, PartitionSpec
        from jax.experimental.shard_map import shard_map
        import concourse.mybir as mybir
        from concourse import bass2jax
        from concourse.bass2jax import _bass_exec_p, install_neuronx_cc_hook

        install_neuronx_cc_hook()
        self.jax = jax
        self.n_cores = n_cores
        partition_name = (
            nc.partition_id_tensor.name if nc.partition_id_tensor else None)
        dbg_name = nc.dbg_addr.name if nc.dbg_addr else None
        in_names, out_names, out_avals, zero_outs = [], [], [], []
        for alloc in nc.m.functions[0].allocations:
            if not isinstance(alloc, mybir.MemoryLocationSet):
                continue
            name = alloc.memorylocations[0].name
            if alloc.kind == "ExternalInput":
                if name not in (partition_name, dbg_name):
                    in_names.append(name)
            elif alloc.kind == "ExternalOutput":
                out_names.append(name)
                shape = tuple(alloc.tensor_shape)
                dtype = mybir.dt.np(alloc.dtype)
                out_avals.append(jax.core.ShapedArray(shape, dtype))
                zero_outs.append(np.zeros(shape, dtype))
        self.in_names, self.out_names = in_names, out_names
        self.out_avals, self.zero_outs = out_avals, zero_outs
        n_params, n_outs = len(in_names), len(out_names)
        all_in_names = list(in_names) + list(out_names)
        if dbg_name is not None:
            all_in_names.append(dbg_name)
        if partition_name is not None:
            all_in_names.append(partition_name)

        def _body(*args):
            operands = list(args)
            if dbg_name is not None:
                operands.append(np.zeros((1, 2), np.uint32))
            if partition_name is not None:
                operands.append(bass2jax.partition_id_tensor())
            outs = _bass_exec_p.bind(
                *operands, out_avals=tuple(out_avals), in_names=tuple(all_in_names),
                out_names=tuple(out_names), lowering_input_output_aliases=(),
                sim_require_finite=False, sim_require_nnan=False, nc=nc)
            return tuple(outs)

        devices = jax.devices()[:n_cores]
        self.mesh = Mesh(np.asarray(devices), ("core",))
        in_specs = (PartitionSpec("core"),) * (n_params + n_outs)
        out_specs = (PartitionSpec("core"),) * n_outs
        self.fn = jax.jit(
            shard_map(_body, mesh=self.mesh, in_specs=in_specs,
                      out_specs=out_specs, check_rep=False),
            keep_unused=True)

    def stage(self, in_maps):
        import jax
        from jax.sharding import PartitionSpec
        n = self.n_cores
        arrs = [np.concatenate([np.asarray(in_maps[c][k]) for c in range(n)], axis=0)
                for k in self.in_names]
        arrs += [np.zeros((n * z.shape[0], *z.shape[1:]), z.dtype) for z in self.zero_outs]
        sh = jax.sharding.NamedSharding(self.mesh, PartitionSpec("core"))
        self._staged = [jax.device_put(a, sh) for a in arrs]

    def run(self):
        outs = self.fn(*self._staged)
        self.jax.block_until_ready(outs)
        return outs

    def results(self, outs):
        n = self.n_cores
        return [
            {name: np.asarray(outs[i]).reshape(n, *self.out_avals[i].shape)[c]
             for i, name in enumerate(self.out_names)}
            for c in range(n)]


# ----------------------------------------------------------------------------
# Entry point
# ----------------------------------------------------------------------------

_PKEYS = ("W0", "b0", "gn0_gamma", "gn0_beta", "gn0_alpha", "gat_W",
          "gat_att_src", "gat_att_dst", "gat_b", "gn_gamma", "gn_beta",
          "gn_alpha", "lin_W", "lin_b")


def _get_runner_and_inmaps(x1, x2, edge_index1, edge_index2, batch1, batch2,
                           params):
    if isinstance(params, dict):
        params = tuple(np.asarray(params[k]) for k in _PKEYS)
    n, f_in = x1.shape
    f = params[0].shape[0]
    f_out = params[-2].shape[0]
    pk1 = prep_branch(x1, edge_index1, batch1)
    pk2 = prep_branch(x2, edge_index2, batch2)
    rq = max(pk1["rq"], pk2["rq"])
    a1, rt = pack_branch(pk1, rq)
    a2, _ = pack_branch(pk2, rq)
    cnt1 = np.maximum(np.bincount(batch1, minlength=G), 1).astype(np.float32)
    cnt2 = np.maximum(np.bincount(batch2, minlength=G), 1).astype(np.float32)
    consts = _make_consts(f, f_out, params, cnt1, cnt2)
    nc = _build_program(pk1["nt"], pk1["nloc_pad"], pk1["nrows"], pk1["rgs"],
                        rq, f, f_out, consts)
    in_maps = []
    for c in range(NCORES):
        m = {}
        for b, a in ((1, a1), (2, a2)):
            m[f"xT{b}"] = a["xT"][c]
            m[f"idx{b}"] = a["idxg"][c]
            m[f"dstf{b}"] = a["dstf"][c]
            m[f"st{b}"] = a["st"][c]
            m[f"bt{b}"] = a["bt"][c]
            m[f"batf{b}"] = a["batf"][c]
            m[f"disf{b}"] = a["disf"][c]
        in_maps.append(m)
    runner = _Runner(nc, NCORES)
    return runner, in_maps, pk1, pk2, f_out


def kernel(x1, x2, edge_index1, edge_index2, batch1, batch2,
           W0, b0, gn0_gamma, gn0_beta, gn0_alpha,
           gat_W, gat_att_src, gat_att_dst, gat_b, gn_gamma, gn_beta, gn_alpha,
           lin_W, lin_b):
    params = tuple(np.asarray(p) for p in (
        W0, b0, gn0_gamma, gn0_beta, gn0_alpha, gat_W, gat_att_src,
        gat_att_dst, gat_b, gn_gamma, gn_beta, gn_alpha, lin_W, lin_b))
    x1 = np.asarray(x1, np.float32)
    x2 = np.asarray(x2, np.float32)
    batch1 = np.asarray(batch1)
    batch2 = np.asarray(batch2)
    runner, in_maps, pk1, pk2, f_out = _get_runner_and_inmaps(
        x1, x2, np.asarray(edge_index1), np.asarray(edge_index2),
        batch1, batch2, params)
    runner.stage(in_maps)
    res = runner.results(runner.run())
    n = x1.shape[0]
    out = np.zeros((2, n, f_out), np.float32)
    for bi, pk in ((0, pk1), (1, pk2)):
        rows = pk["t_of"] * P + pk["slot_of"]
        allout = np.stack([res[c][f"out{bi + 1}"] for c in range(NCORES)])
        out[bi] = allout[pk["core_of"], rows]
    return out
